# revision 18
# baseline (speedup 1.0000x reference)
"""Trainium2 Bass kernel for dual-branch (low-rank + full-rank) self-attention.

Math (per batch b, head h):
  q = x @ Wq_cat[h].T   (N, 224)   224 = 64 (lr) + 160 (full)
  scoresT[m, n] = sum_d K[m, d] Q[n, d]           (keys m on partitions)
  expT = exp(SCALE * scoresT)                     (no max subtraction; f32 psum)
  xav[d, n] = sum_m Vaug[m, d] expT[m, n]         Vaug has a ones column ->
                                                  psum row 96 of hi = denom
  recip = 1/denom (f32 approx-fast row), partition_broadcast on gpsimd,
  and the AV psum->sbuf drains multiply by it (fused normalize) into a
  PACKED xav:
    packed row id: rid(h, d) = 128h + d        (d < 128, "lo")
                   rid(h, d) = 1024 + 96h + (d-128)   (d >= 128, "hi")
  so the output projection contracts exactly 14 full 128-row chunks
  (14 matmuls per output tile instead of 16); bias is added by the
  scalar-engine drain (per-partition bias vector).

Sharding: data-parallel, 2 batches per core across 8 cores. No collectives.
All matmuls bf16 with f32 PSUM accumulation; softmax in f32 psum.
"""

import os
import sys

sys.path.insert(0, "/opt/trn_rl_repo")

import numpy as np
import ml_dtypes

import concourse.bass as bass
import concourse.mybir as mybir
import concourse.tile as tile
from concourse import bacc
from concourse.bass_utils import run_bass_kernel_spmd

# problem constants (hardcoded per spec)
B, N, C = 16, 1024, 1280
HEADS = 8
RANK = 64
DIM_HEAD = 160
DH = RANK + DIM_HEAD          # 224 concat head dim
SCALE = DIM_HEAD ** (-0.5)
NCORES = 8
BL = B // NCORES              # batches per core = 2
CK = C // 128                 # 10 contraction chunks over C
GROUPS = 2                    # head groups per core pass
HG = HEADS // GROUPS          # 4 heads per group
MC = N // 128                 # 8 key chunks
NT = N // 512                 # 2 query-column tiles
PK = (HEADS * DH) // 128      # 14 packed out-proj contraction chunks

BF16 = mybir.dt.bfloat16
F32 = mybir.dt.float32


def _hi_frags(h):
    """Fragment list for copying AV-hi psum rows 0:96 of head h into the
    packed xav chunks. Returns [(src_base, chunk, dst_base, rows)] where
    every operand window is legal: base 0 spans freely, other bases are
    32-aligned with rows <= 32."""
    frags = []
    start = 1024 + 96 * h
    src = 0
    while src < 96:
        rid = start + src
        chunk, db = divmod(rid, 128)
        rows = min(96 - src, 128 - db)
        if db != 0 or src != 0:
            rows = min(rows, 32)
        frags.append((src, chunk, db, rows))
        src += rows
    return frags


# out-proj accumulation order: the chunks fed by the last head's drains
# (7 = h7 lo, 13 = h5-7 hi) go last so the tail's final projection has
# maximal PE work ready before those drains land
PK_ORDER = [0, 1, 2, 3, 8, 9, 10, 4, 5, 6, 11, 12, 7, 13]


def build_bass():
    nc = bacc.Bacc("TRN2", target_bir_lowering=False, debug=False,
                   num_devices=NCORES)

    def din(name, shape, dt=BF16):
        return nc.dram_tensor(name, shape, dt, kind="ExternalInput").ap()

    xt_d = din("xt", [128, CK, BL * N])                 # x transposed, c-major
    wq_lo_d = din("wq_lo", [GROUPS, 128, CK, HG * 128])
    wq_hi_d = din("wq_hi", [GROUPS, 128, CK, HG * 96])
    wk_lo_d = din("wk_lo", [GROUPS, 128, CK, HG * 128])
    wk_hi_d = din("wk_hi", [GROUPS, 128, CK, HG * 96])
    wv_lo_d = din("wv_lo", [GROUPS, 128, CK, HG * 128])
    wv_hi_d = din("wv_hi", [GROUPS, 128, CK, HG * 97])  # 97th col zero (ones col)
    wo_pk_d = din("wo_pk", [PK, 128, C])                # packed out-proj weights
    bias_d = din("bias", [128, CK], mybir.dt.float32)   # bias[ct*128+p] at [p, ct]
    out_d = nc.dram_tensor("out", [BL, CK, 128, N], BF16, kind="ExternalOutput").ap()

    with tile.TileContext(nc) as tc:
        with (
            tc.tile_pool(name="xtp", bufs=1) as xtp,
            tc.tile_pool(name="wp", bufs=5) as wp,
            tc.tile_pool(name="wop", bufs=PK + 1) as wop,
            tc.tile_pool(name="qkvp", bufs=1) as qkvp,
            tc.tile_pool(name="xavp", bufs=1) as xavp,
            tc.tile_pool(name="expp", bufs=1) as expp,
            tc.tile_pool(name="nrmp", bufs=2) as nrmp,
            tc.tile_pool(name="outp", bufs=3) as outp,
            tc.tile_pool(name="psp", bufs=4, space="PSUM") as psp,
            tc.tile_pool(name="psavp", bufs=4, space="PSUM") as psavp,
        ):
            def ps_tile():
                # general-purpose matmul accumulators (scores/proj/out-proj)
                return psp.tile([128, 512], F32, tag="mm", name="ps")

            def ps_av_tile():
                # AV accumulators — separate slots so the normalize/drain
                # chain never blocks the next head's scores
                return psavp.tile([128, 512], F32, tag="av", name="ps_av")

            wo_t = []          # persistent packed out-proj weight tiles
            bias_t = None
            pending_out = []

            def emit_out_proj():
                # output projection (c on partitions; host untransposes)
                ob, oxav = pending_out.pop(0)
                for ct in range(CK):
                    for nt in range(NT):
                        ps_o = ps_tile()
                        for i, pk in enumerate(PK_ORDER):
                            nc.tensor.matmul(
                                ps_o[:],
                                wo_t[pk][:, ct * 128:(ct + 1) * 128],
                                oxav[:, pk, nt * 512:(nt + 1) * 512],
                                start=(i == 0), stop=(i == PK - 1))
                        ot = outp.tile([128, 512], BF16, tag="ot", name="ot")
                        nc.scalar.activation(
                            ot[:], ps_o[:],
                            mybir.ActivationFunctionType.Identity,
                            bias=bias_t[:, ct:ct + 1], scale=1.0)
                        nc.sync.dma_start(
                            out_d[ob, ct, :, nt * 512:(nt + 1) * 512], ot[:])

            for b in range(BL):
                xt = xtp.tile([128, CK, N], BF16, tag="xt")
                if b > 0:
                    for co in range(CK):
                        nc.sync.dma_start(xt[:, co, :],
                                          xt_d[:, co, b * N:(b + 1) * N])

                xav = xavp.tile([128, PK, N], BF16, tag="xav")

                for g in range(GROUPS):
                    # ---- stream this group's projection weights ----
                    def slab(width):
                        return wp.tile([128, CK, width], BF16, tag="wslab",
                                       name="wslab")
                    w_qlo = slab(HG * 128)
                    w_qhi = slab(HG * 96)
                    w_klo = slab(HG * 128)
                    w_khi = slab(HG * 96)
                    w_vlo = slab(HG * 128)
                    w_vhi = slab(HG * 97)

                    def load_chunk(t, dram, co):
                        nc.sync.dma_start(t[:, co, :], dram[g, :, co, :])

                    qk_slabs = [(w_qlo, wq_lo_d), (w_qhi, wq_hi_d),
                                (w_klo, wk_lo_d), (w_khi, wk_hi_d)]
                    v_slabs = [(w_vlo, wv_lo_d), (w_vhi, wv_hi_d)]
                    if b == 0 and g == 0:
                        # startup: the critical stream is xt + wq_lo — issue
                        # those interleaved first, the rest behind them
                        for co in range(CK):
                            load_chunk(w_qlo, wq_lo_d, co)
                            nc.sync.dma_start(xt[:, co, :],
                                              xt_d[:, co, 0:N])
                        for co in range(CK):
                            for t, dram in qk_slabs[1:]:
                                load_chunk(t, dram, co)
                        for co in range(CK):
                            for t, dram in v_slabs:
                                load_chunk(t, dram, co)
                    else:
                        for t, dram in qk_slabs + v_slabs:
                            for co in range(CK):
                                load_chunk(t, dram, co)

                    # ---- projections: qT/kT (head-dim on partitions) ----
                    qt_lo = qkvp.tile([128, HG, N], BF16, tag="qt_lo")
                    qt_hi = qkvp.tile([128, HG, N], BF16, tag="qt_hi")
                    kt_lo = qkvp.tile([128, HG, N], BF16, tag="kt_lo")
                    kt_hi = qkvp.tile([128, HG, N], BF16, tag="kt_hi")

                    # hi chunks of 4 heads (4x96=384 rows) pack into 3 full
                    # M=128 matmuls; fragments unmixed by the psum->sbuf
                    # copies (pieces respect the 32-aligned partition-window
                    # rule: base 0 may span freely, other bases max 32 rows)
                    HI_FRAGS = {  # ch -> [(hl, src_base, dst_base, rows)]
                        0: [(0, 0, 0, 96), (1, 96, 0, 32)],
                        1: [(1, 0, 32, 32), (1, 32, 64, 32),
                            (2, 64, 0, 32), (2, 96, 32, 32)],
                        2: [(2, 0, 64, 32), (3, 32, 0, 32),
                            (3, 64, 32, 32), (3, 96, 64, 32)],
                    }
                    def proj_lo(wlo, tlo, co_outer):
                        units = [(hl, nt) for hl in range(HG)
                                 for nt in range(NT)]
                        if co_outer:
                            # all 8 psum banks accumulate at once so each
                            # arriving DMA chunk feeds 8 matmuls immediately
                            # (otherwise the single open chain is DMA-paced
                            # during startup)
                            pss = ([ps_tile() for _ in range(4)]
                                   + [ps_av_tile() for _ in range(4)])
                            for co in range(CK):
                                for i, (hl, nt) in enumerate(units):
                                    nc.tensor.matmul(
                                        pss[i][:],
                                        wlo[:, co, hl * 128:(hl + 1) * 128],
                                        xt[:, co, nt * 512:(nt + 1) * 512],
                                        start=(co == 0), stop=(co == CK - 1))
                            for i, (hl, nt) in enumerate(units):
                                nc.vector.tensor_copy(
                                    tlo[:, hl, nt * 512:(nt + 1) * 512],
                                    pss[i][:])
                        else:
                            for (hl, nt) in units:
                                ps = ps_tile()
                                for co in range(CK):
                                    nc.tensor.matmul(
                                        ps[:],
                                        wlo[:, co, hl * 128:(hl + 1) * 128],
                                        xt[:, co, nt * 512:(nt + 1) * 512],
                                        start=(co == 0), stop=(co == CK - 1))
                                nc.vector.tensor_copy(
                                    tlo[:, hl, nt * 512:(nt + 1) * 512], ps[:])

                    def proj_hi(whi, thi, co_outer):
                        units = [(ch, nt) for ch in range(3)
                                 for nt in range(NT)]

                        def drain(ps, ch, nt):
                            for (hl, sb, db, rows) in HI_FRAGS[ch]:
                                nc.vector.tensor_copy(
                                    thi[db:db + rows, hl,
                                        nt * 512:(nt + 1) * 512],
                                    ps[sb:sb + rows, :])
                        if co_outer:
                            # lean on the AV pool here so the following
                            # K-lo projections find free "mm" slots instead
                            # of queuing behind this phase's drain burst
                            pss = ([ps_tile() for _ in range(2)]
                                   + [ps_av_tile() for _ in range(4)])
                            for co in range(CK):
                                for i, (ch, nt) in enumerate(units):
                                    nc.tensor.matmul(
                                        pss[i][:],
                                        whi[:, co, ch * 128:(ch + 1) * 128],
                                        xt[:, co, nt * 512:(nt + 1) * 512],
                                        start=(co == 0), stop=(co == CK - 1))
                            for i, (ch, nt) in enumerate(units):
                                drain(pss[i], ch, nt)
                        else:
                            for (ch, nt) in units:
                                ps = ps_tile()
                                for co in range(CK):
                                    nc.tensor.matmul(
                                        ps[:],
                                        whi[:, co, ch * 128:(ch + 1) * 128],
                                        xt[:, co, nt * 512:(nt + 1) * 512],
                                        start=(co == 0), stop=(co == CK - 1))
                                drain(ps, ch, nt)

                    startup = (b == 0 and g == 0)
                    proj_lo(w_qlo, qt_lo, startup)
                    proj_hi(w_qhi, qt_hi, startup)
                    proj_lo(w_klo, kt_lo, False)
                    proj_hi(w_khi, kt_hi, False)

                    # ---- V projection: natural layout (keys on partitions) ----
                    v_lo = qkvp.tile([128, MC, HG * 128], BF16, tag="v_lo")
                    v_hi = qkvp.tile([128, MC, HG * 97], BF16, tag="v_hi")
                    for mc in range(MC):
                        ps_l = ps_tile()
                        ps_h = ps_tile()
                        for co in range(CK):
                            nc.tensor.matmul(
                                ps_l[:],
                                xt[:, co, mc * 128:(mc + 1) * 128],
                                w_vlo[:, co, :],
                                start=(co == 0), stop=(co == CK - 1))
                        for co in range(CK):
                            nc.tensor.matmul(
                                ps_h[:, 0:HG * 97],
                                xt[:, co, mc * 128:(mc + 1) * 128],
                                w_vhi[:, co, :],
                                start=(co == 0), stop=(co == CK - 1))
                        nc.vector.tensor_copy(v_lo[:, mc, :], ps_l[:])
                        nc.vector.tensor_copy(v_hi[:, mc, :], ps_h[:, 0:HG * 97])
                    # ones column per head (softmax denominator row source)
                    for hl in range(HG):
                        nc.gpsimd.memset(v_hi[:, :, hl * 97 + 96], 1.0)

                    # persistent out-proj weights: emit once, during b0/g0
                    # compute (keeps the startup DMA window clear)
                    if b == 0 and g == 0:
                        for pk in range(PK):
                            wt = wop.tile([128, C], BF16, tag="wo", name="wo")
                            nc.sync.dma_start(wt[:], wo_pk_d[pk])
                            wo_t.append(wt)
                        bias_t = wop.tile([128, CK], F32, tag="bias",
                                          name="bias")
                        nc.sync.dma_start(bias_t[:], bias_d)

                    # previous batch's output projection: emitted while this
                    # group's dense matmuls keep the PE busy
                    if g == 0 and pending_out:
                        emit_out_proj()

                    # ---- attention per head: AV + fused normalize drains ----
                    for hl in range(HG):
                        h_abs = g * HG + hl
                        for nt in range(NT):
                            expt = expp.tile([128, MC * 512], BF16, tag="expt")
                            for mc in range(MC):
                                ps_sc = ps_tile()
                                nc.tensor.matmul(
                                    ps_sc[:],
                                    kt_lo[:, hl, mc * 128:(mc + 1) * 128],
                                    qt_lo[:, hl, nt * 512:(nt + 1) * 512],
                                    start=True, stop=False)
                                nc.tensor.matmul(
                                    ps_sc[:],
                                    kt_hi[0:96, hl, mc * 128:(mc + 1) * 128],
                                    qt_hi[0:96, hl, nt * 512:(nt + 1) * 512],
                                    start=False, stop=True)
                                nc.scalar.activation(
                                    expt[:, mc * 512:(mc + 1) * 512],
                                    ps_sc[:],
                                    mybir.ActivationFunctionType.Exp,
                                    scale=SCALE)
                            ps_alo = ps_av_tile()
                            ps_ahi = ps_av_tile()
                            for mc in range(MC):
                                nc.tensor.matmul(
                                    ps_alo[:],
                                    v_lo[:, mc, hl * 128:(hl + 1) * 128],
                                    expt[:, mc * 512:(mc + 1) * 512],
                                    start=(mc == 0), stop=(mc == MC - 1))
                            for mc in range(MC):
                                nc.tensor.matmul(
                                    ps_ahi[0:97, :],
                                    v_hi[:, mc, hl * 97:(hl + 1) * 97],
                                    expt[:, mc * 512:(mc + 1) * 512],
                                    start=(mc == 0), stop=(mc == MC - 1))
                            # denom (psum row 96) -> partition-0 sbuf row
                            # (custom-DVE ops ignore the AP partition
                            # offset, so recip can't read row 96 directly)
                            # -> f32 approx recip (denoms are sums of exps,
                            # far from the undefined edge cases) -> gpsimd
                            # broadcast -> fused normalize drains
                            rraw = nrmp.tile([1, 512], F32, tag="rraw",
                                             name="rraw")
                            nc.vector.tensor_copy(rraw[:], ps_ahi[96:97, :])
                            rrow = nrmp.tile([1, 512], F32, tag="rrow",
                                             name="rrow")
                            nc.vector.reciprocal_approx_fast(
                                rrow[:], rraw[:])
                            bc = nrmp.tile([128, 512], F32, tag="bc",
                                           name="bc")
                            nc.gpsimd.partition_broadcast(bc[:], rrow[:])
                            ntsl = slice(nt * 512, (nt + 1) * 512)
                            nc.vector.tensor_tensor(
                                xav[:, h_abs, ntsl], ps_alo[:], bc[:],
                                mybir.AluOpType.mult)
                            for (sb, ch, db, rows) in _hi_frags(h_abs):
                                nc.vector.tensor_tensor(
                                    xav[db:db + rows, ch, ntsl],
                                    ps_ahi[sb:sb + rows, :],
                                    bc[0:rows, :],
                                    mybir.AluOpType.mult)

                pending_out.append((b, xav))

            while pending_out:
                emit_out_proj()

    nc.compile()
    return nc


def _prep_weights(Wq_lr, Wk_lr, Wv_lr, Wout_lr, Wq_full, Wk_full, Wv_full,
                  Wout_full, b_out_full):
    """Host-side weight concat/transpose into device layouts (bf16)."""
    bf16 = ml_dtypes.bfloat16

    def cat_heads(W_lr, W_full):
        # -> (H, 224, C)
        lr = W_lr.reshape(HEADS, RANK, C)
        fl = W_full.reshape(HEADS, DIM_HEAD, C)
        return np.concatenate([lr, fl], axis=1)

    def slab_lo(Wcat):
        # (H,224,C) -> per group [G, 128(p), CK, HG*128] with layout
        # [g][p, co, hl*128+j] = Wcat[g*HG+hl, j, co*128+p]
        A = Wcat[:, :128, :].reshape(GROUPS, HG, 128, CK, 128)
        return np.ascontiguousarray(A.transpose(0, 4, 3, 1, 2)
                                    .reshape(GROUPS, 128, CK, HG * 128)
                                    ).astype(bf16)

    def slab_hi(Wcat, width, pad_to=None):
        A = Wcat[:, 128:224, :].reshape(GROUPS, HG, 96, CK, 128)
        A = A.transpose(0, 4, 3, 1, 2)  # (G, p, co, hl, 96)
        if pad_to is not None:
            pad = np.zeros(A.shape[:-1] + (pad_to - 96,), A.dtype)
            A = np.concatenate([A, pad], axis=-1)
            width = pad_to
        return np.ascontiguousarray(
            A.reshape(GROUPS, 128, CK, HG * width)).astype(bf16)

    Wq_cat = cat_heads(Wq_lr, Wq_full)
    Wk_cat = cat_heads(Wk_lr, Wk_full)
    Wv_cat = cat_heads(Wv_lr, Wv_full)

    # output projection, packed: Wo_cat (H, 224, C) with
    # Wo_cat[h, d, c] = Wout_cat[c, h*224+d]; packed row id rid(h, d)
    Wo_lr = Wout_lr.reshape(C, HEADS, RANK)
    Wo_fl = Wout_full.reshape(C, HEADS, DIM_HEAD)
    Wo_cat = np.concatenate([Wo_lr, Wo_fl], axis=2).transpose(1, 2, 0)  # (H,224,C)
    wo_pk = np.empty((HEADS * DH, C), np.float32)
    for h in range(HEADS):
        wo_pk[128 * h:128 * (h + 1)] = Wo_cat[h, :128, :]
        wo_pk[1024 + 96 * h:1024 + 96 * (h + 1)] = Wo_cat[h, 128:, :]
    wo_pk = wo_pk.reshape(PK, 128, C).astype(bf16)

    bias = np.ascontiguousarray(
        b_out_full.reshape(CK, 128).T).astype(np.float32)  # [128, CK]

    return {
        "wq_lo": slab_lo(Wq_cat), "wq_hi": slab_hi(Wq_cat, 96),
        "wk_lo": slab_lo(Wk_cat), "wk_hi": slab_hi(Wk_cat, 96),
        "wv_lo": slab_lo(Wv_cat), "wv_hi": slab_hi(Wv_cat, 96, pad_to=97),
        "wo_pk": wo_pk, "bias": bias,
    }


def _prep_xt(hs_core):
    # (BL, N, C) f32 -> [128, CK, BL*N] bf16, xt[p, co, b*N+n] = x[b, n, co*128+p]
    X = hs_core.reshape(BL * N, CK, 128).transpose(2, 1, 0)
    return np.ascontiguousarray(X).astype(ml_dtypes.bfloat16)


_NC_CACHE = {}


def get_nc():
    if "nc" not in _NC_CACHE:
        _NC_CACHE["nc"] = build_bass()
    return _NC_CACHE["nc"]


def kernel(hidden_states, Wq_lr, Wk_lr, Wv_lr, Wout_lr,
           Wq_full, Wk_full, Wv_full, Wout_full, b_out_full):
    hidden_states = np.asarray(hidden_states, np.float32)
    weights = _prep_weights(
        np.asarray(Wq_lr, np.float32), np.asarray(Wk_lr, np.float32),
        np.asarray(Wv_lr, np.float32), np.asarray(Wout_lr, np.float32),
        np.asarray(Wq_full, np.float32), np.asarray(Wk_full, np.float32),
        np.asarray(Wv_full, np.float32), np.asarray(Wout_full, np.float32),
        np.asarray(b_out_full, np.float32))

    in_maps = []
    for c in range(NCORES):
        m = dict(weights)
        m["xt"] = _prep_xt(hidden_states[c * BL:(c + 1) * BL])
        in_maps.append(m)

    nc = get_nc()
    results = run_bass_kernel_spmd(nc, in_maps, core_ids=list(range(NCORES))).results

    out = np.empty((B, N, C), np.float32)
    for c in range(NCORES):
        o = results[c]["out"].astype(np.float32)  # (BL, CK, 128, N)
        out[c * BL:(c + 1) * BL] = (
            o.transpose(0, 3, 1, 2).reshape(BL, N, C))
    return out


if __name__ == "__main__":
    nc = get_nc()
    print("built + compiled OK")


# revision 20
# speedup vs baseline: 1.0031x; 1.0031x over previous
"""Trainium2 Bass kernel for dual-branch (low-rank + full-rank) self-attention.

Math (per batch b, head h):
  q = x @ Wq_cat[h].T   (N, 224)   224 = 64 (lr) + 160 (full)
  scoresT[m, n] = sum_d K[m, d] Q[n, d]           (keys m on partitions)
  expT = exp(SCALE * scoresT)                     (no max subtraction; f32 psum)
  xav[d, n] = sum_m Vaug[m, d] expT[m, n]         Vaug has a ones column ->
                                                  psum row 96 of hi = denom
  recip = 1/denom (f32 approx-fast row), partition_broadcast on gpsimd,
  and the AV psum->sbuf drains multiply by it (fused normalize) into a
  PACKED xav:
    packed row id: rid(h, d) = 128h + d        (d < 128, "lo")
                   rid(h, d) = 1024 + 96h + (d-128)   (d >= 128, "hi")
  so the output projection contracts exactly 14 full 128-row chunks
  (14 matmuls per output tile instead of 16); bias is added by the
  scalar-engine drain (per-partition bias vector).

Sharding: data-parallel, 2 batches per core across 8 cores. No collectives.
All matmuls bf16 with f32 PSUM accumulation; softmax in f32 psum.
"""

import os
import sys

sys.path.insert(0, "/opt/trn_rl_repo")

import numpy as np
import ml_dtypes

import concourse.bass as bass
import concourse.mybir as mybir
import concourse.tile as tile
from concourse import bacc
from concourse.bass_utils import run_bass_kernel_spmd

# problem constants (hardcoded per spec)
B, N, C = 16, 1024, 1280
HEADS = 8
RANK = 64
DIM_HEAD = 160
DH = RANK + DIM_HEAD          # 224 concat head dim
SCALE = DIM_HEAD ** (-0.5)
NCORES = 8
BL = B // NCORES              # batches per core = 2
CK = C // 128                 # 10 contraction chunks over C
GROUPS = 2                    # head groups per core pass
HG = HEADS // GROUPS          # 4 heads per group
MC = N // 128                 # 8 key chunks
NT = N // 512                 # 2 query-column tiles
PK = (HEADS * DH) // 128      # 14 packed out-proj contraction chunks

BF16 = mybir.dt.bfloat16
F32 = mybir.dt.float32


def _hi_frags(h):
    """Fragment list for copying AV-hi psum rows 0:96 of head h into the
    packed xav chunks. Returns [(src_base, chunk, dst_base, rows)] where
    every operand window is legal: base 0 spans freely, other bases are
    32-aligned with rows <= 32."""
    frags = []
    start = 1024 + 96 * h
    src = 0
    while src < 96:
        rid = start + src
        chunk, db = divmod(rid, 128)
        rows = min(96 - src, 128 - db)
        if db != 0 or src != 0:
            rows = min(rows, 32)
        frags.append((src, chunk, db, rows))
        src += rows
    return frags


# out-proj accumulation order: the chunks fed by the last head's drains
# (7 = h7 lo, 13 = h5-7 hi) go last so the tail's final projection has
# maximal PE work ready before those drains land
PK_ORDER = [0, 1, 2, 3, 8, 9, 10, 4, 5, 6, 11, 12, 7, 13]


def build_bass():
    nc = bacc.Bacc("TRN2", target_bir_lowering=False, debug=False,
                   num_devices=NCORES)

    def din(name, shape, dt=BF16):
        return nc.dram_tensor(name, shape, dt, kind="ExternalInput").ap()

    xt_d = din("xt", [128, CK, BL * N])                 # x transposed, c-major
    wq_lo_d = din("wq_lo", [GROUPS, 128, CK, HG * 128])
    wq_hi_d = din("wq_hi", [GROUPS, 128, CK, HG * 96])
    wk_lo_d = din("wk_lo", [GROUPS, 128, CK, HG * 128])
    wk_hi_d = din("wk_hi", [GROUPS, 128, CK, HG * 96])
    wv_lo_d = din("wv_lo", [GROUPS, 128, CK, HG * 128])
    wv_hi_d = din("wv_hi", [GROUPS, 128, CK, HG * 97])  # 97th col zero (ones col)
    wo_pk_d = din("wo_pk", [PK, 128, C])                # packed out-proj weights
    bias_d = din("bias", [128, CK], mybir.dt.float32)   # bias[ct*128+p] at [p, ct]
    out_d = nc.dram_tensor("out", [BL, CK, 128, N], BF16, kind="ExternalOutput").ap()

    with tile.TileContext(nc) as tc:
        with (
            tc.tile_pool(name="xtp", bufs=1) as xtp,
            tc.tile_pool(name="wp", bufs=5) as wp,
            tc.tile_pool(name="wop", bufs=PK + 1) as wop,
            tc.tile_pool(name="qkvp", bufs=1) as qkvp,
            tc.tile_pool(name="xavp", bufs=1) as xavp,
            tc.tile_pool(name="expp", bufs=1) as expp,
            tc.tile_pool(name="nrmp", bufs=2) as nrmp,
            tc.tile_pool(name="outp", bufs=3) as outp,
            tc.tile_pool(name="psp", bufs=4, space="PSUM") as psp,
            tc.tile_pool(name="psavp", bufs=4, space="PSUM") as psavp,
        ):
            def ps_tile():
                # general-purpose matmul accumulators (scores/proj/out-proj)
                return psp.tile([128, 512], F32, tag="mm", name="ps")

            def ps_av_tile():
                # AV accumulators — separate slots so the normalize/drain
                # chain never blocks the next head's scores
                return psavp.tile([128, 512], F32, tag="av", name="ps_av")

            wo_t = []          # persistent packed out-proj weight tiles
            bias_t = None
            pending_out = []

            def emit_out_proj():
                # output projection (c on partitions; host untransposes)
                ob, oxav = pending_out.pop(0)
                for ct in range(CK):
                    for nt in range(NT):
                        ps_o = ps_tile()
                        for i, pk in enumerate(PK_ORDER):
                            nc.tensor.matmul(
                                ps_o[:],
                                wo_t[pk][:, ct * 128:(ct + 1) * 128],
                                oxav[:, pk, nt * 512:(nt + 1) * 512],
                                start=(i == 0), stop=(i == PK - 1))
                        ot = outp.tile([128, 512], BF16, tag="ot", name="ot")
                        nc.scalar.activation(
                            ot[:], ps_o[:],
                            mybir.ActivationFunctionType.Identity,
                            bias=bias_t[:, ct:ct + 1], scale=1.0)
                        nc.sync.dma_start(
                            out_d[ob, ct, :, nt * 512:(nt + 1) * 512], ot[:])

            for b in range(BL):
                xt = xtp.tile([128, CK, N], BF16, tag="xt")
                if b > 0:
                    for co in range(CK):
                        nc.sync.dma_start(xt[:, co, :],
                                          xt_d[:, co, b * N:(b + 1) * N])

                xav = xavp.tile([128, PK, N], BF16, tag="xav")

                for g in range(GROUPS):
                    # ---- stream this group's projection weights ----
                    def slab(width):
                        return wp.tile([128, CK, width], BF16, tag="wslab",
                                       name="wslab")
                    w_qlo = slab(HG * 128)
                    w_qhi = slab(HG * 96)
                    w_klo = slab(HG * 128)
                    w_khi = slab(HG * 96)
                    w_vlo = slab(HG * 128)
                    w_vhi = slab(HG * 97)

                    def load_chunk(t, dram, co):
                        nc.sync.dma_start(t[:, co, :], dram[g, :, co, :])

                    qk_slabs = [(w_qlo, wq_lo_d), (w_qhi, wq_hi_d),
                                (w_klo, wk_lo_d), (w_khi, wk_hi_d)]
                    v_slabs = [(w_vlo, wv_lo_d), (w_vhi, wv_hi_d)]
                    if b == 0 and g == 0:
                        # startup: the critical stream is xt + wq_lo — issue
                        # those interleaved first, the rest behind them
                        for co in range(CK):
                            load_chunk(w_qlo, wq_lo_d, co)
                            nc.sync.dma_start(xt[:, co, :],
                                              xt_d[:, co, 0:N])
                        for co in range(CK):
                            for t, dram in qk_slabs[1:]:
                                load_chunk(t, dram, co)
                        for co in range(CK):
                            for t, dram in v_slabs:
                                load_chunk(t, dram, co)
                    else:
                        for t, dram in qk_slabs + v_slabs:
                            for co in range(CK):
                                load_chunk(t, dram, co)

                    # ---- projections: qT/kT (head-dim on partitions) ----
                    qt_lo = qkvp.tile([128, HG, N], BF16, tag="qt_lo")
                    qt_hi = qkvp.tile([128, HG, N], BF16, tag="qt_hi")
                    kt_lo = qkvp.tile([128, HG, N], BF16, tag="kt_lo")
                    kt_hi = qkvp.tile([128, HG, N], BF16, tag="kt_hi")

                    # hi chunks of 4 heads (4x96=384 rows) pack into 3 full
                    # M=128 matmuls; fragments unmixed by the psum->sbuf
                    # copies (pieces respect the 32-aligned partition-window
                    # rule: base 0 may span freely, other bases max 32 rows)
                    HI_FRAGS = {  # ch -> [(hl, src_base, dst_base, rows)]
                        0: [(0, 0, 0, 96), (1, 96, 0, 32)],
                        1: [(1, 0, 32, 32), (1, 32, 64, 32),
                            (2, 64, 0, 32), (2, 96, 32, 32)],
                        2: [(2, 0, 64, 32), (3, 32, 0, 32),
                            (3, 64, 32, 32), (3, 96, 64, 32)],
                    }
                    def proj_lo(wlo, tlo, co_outer):
                        units = [(hl, nt) for hl in range(HG)
                                 for nt in range(NT)]
                        if co_outer:
                            # all 8 psum banks accumulate at once so each
                            # arriving DMA chunk feeds 8 matmuls immediately
                            # (otherwise the single open chain is DMA-paced
                            # during startup)
                            pss = ([ps_tile() for _ in range(4)]
                                   + [ps_av_tile() for _ in range(4)])
                            for co in range(CK):
                                for i, (hl, nt) in enumerate(units):
                                    nc.tensor.matmul(
                                        pss[i][:],
                                        wlo[:, co, hl * 128:(hl + 1) * 128],
                                        xt[:, co, nt * 512:(nt + 1) * 512],
                                        start=(co == 0), stop=(co == CK - 1))
                            for i, (hl, nt) in enumerate(units):
                                # scalar engine: it is idle until the first
                                # exps (~60us in), and this keeps the DVE
                                # queue clear of the co-outer drain burst so
                                # later projections get psum slots promptly
                                nc.scalar.copy(
                                    tlo[:, hl, nt * 512:(nt + 1) * 512],
                                    pss[i][:])
                        else:
                            for (hl, nt) in units:
                                ps = ps_tile()
                                for co in range(CK):
                                    nc.tensor.matmul(
                                        ps[:],
                                        wlo[:, co, hl * 128:(hl + 1) * 128],
                                        xt[:, co, nt * 512:(nt + 1) * 512],
                                        start=(co == 0), stop=(co == CK - 1))
                                nc.vector.tensor_copy(
                                    tlo[:, hl, nt * 512:(nt + 1) * 512], ps[:])

                    def proj_hi(whi, thi, co_outer):
                        units = [(ch, nt) for ch in range(3)
                                 for nt in range(NT)]

                        def drain(ps, ch, nt):
                            for (hl, sb, db, rows) in HI_FRAGS[ch]:
                                nc.vector.tensor_copy(
                                    thi[db:db + rows, hl,
                                        nt * 512:(nt + 1) * 512],
                                    ps[sb:sb + rows, :])
                        if co_outer:
                            pss = ([ps_tile() for _ in range(4)]
                                   + [ps_av_tile() for _ in range(2)])
                            for co in range(CK):
                                for i, (ch, nt) in enumerate(units):
                                    nc.tensor.matmul(
                                        pss[i][:],
                                        whi[:, co, ch * 128:(ch + 1) * 128],
                                        xt[:, co, nt * 512:(nt + 1) * 512],
                                        start=(co == 0), stop=(co == CK - 1))
                            for i, (ch, nt) in enumerate(units):
                                drain(pss[i], ch, nt)
                        else:
                            for (ch, nt) in units:
                                ps = ps_tile()
                                for co in range(CK):
                                    nc.tensor.matmul(
                                        ps[:],
                                        whi[:, co, ch * 128:(ch + 1) * 128],
                                        xt[:, co, nt * 512:(nt + 1) * 512],
                                        start=(co == 0), stop=(co == CK - 1))
                                drain(ps, ch, nt)

                    startup = (b == 0 and g == 0)
                    proj_lo(w_qlo, qt_lo, startup)
                    proj_hi(w_qhi, qt_hi, startup)
                    proj_lo(w_klo, kt_lo, False)
                    proj_hi(w_khi, kt_hi, False)

                    # ---- V projection: natural layout (keys on partitions) ----
                    v_lo = qkvp.tile([128, MC, HG * 128], BF16, tag="v_lo")
                    v_hi = qkvp.tile([128, MC, HG * 97], BF16, tag="v_hi")
                    for mc in range(MC):
                        ps_l = ps_tile()
                        ps_h = ps_tile()
                        for co in range(CK):
                            nc.tensor.matmul(
                                ps_l[:],
                                xt[:, co, mc * 128:(mc + 1) * 128],
                                w_vlo[:, co, :],
                                start=(co == 0), stop=(co == CK - 1))
                        for co in range(CK):
                            nc.tensor.matmul(
                                ps_h[:, 0:HG * 97],
                                xt[:, co, mc * 128:(mc + 1) * 128],
                                w_vhi[:, co, :],
                                start=(co == 0), stop=(co == CK - 1))
                        nc.vector.tensor_copy(v_lo[:, mc, :], ps_l[:])
                        nc.vector.tensor_copy(v_hi[:, mc, :], ps_h[:, 0:HG * 97])
                    # ones column per head (softmax denominator row source)
                    for hl in range(HG):
                        nc.gpsimd.memset(v_hi[:, :, hl * 97 + 96], 1.0)

                    # persistent out-proj weights: emit once, during b0/g0
                    # compute (keeps the startup DMA window clear)
                    if b == 0 and g == 0:
                        for pk in range(PK):
                            wt = wop.tile([128, C], BF16, tag="wo", name="wo")
                            nc.sync.dma_start(wt[:], wo_pk_d[pk])
                            wo_t.append(wt)
                        bias_t = wop.tile([128, CK], F32, tag="bias",
                                          name="bias")
                        nc.sync.dma_start(bias_t[:], bias_d)

                    # previous batch's output projection: emitted while this
                    # group's dense matmuls keep the PE busy
                    if g == 0 and pending_out:
                        emit_out_proj()

                    # ---- attention per head: AV + fused normalize drains ----
                    for hl in range(HG):
                        h_abs = g * HG + hl
                        for nt in range(NT):
                            expt = expp.tile([128, MC * 512], BF16, tag="expt")
                            for mc in range(MC):
                                ps_sc = ps_tile()
                                nc.tensor.matmul(
                                    ps_sc[:],
                                    kt_lo[:, hl, mc * 128:(mc + 1) * 128],
                                    qt_lo[:, hl, nt * 512:(nt + 1) * 512],
                                    start=True, stop=False)
                                nc.tensor.matmul(
                                    ps_sc[:],
                                    kt_hi[0:96, hl, mc * 128:(mc + 1) * 128],
                                    qt_hi[0:96, hl, nt * 512:(nt + 1) * 512],
                                    start=False, stop=True)
                                nc.scalar.activation(
                                    expt[:, mc * 512:(mc + 1) * 512],
                                    ps_sc[:],
                                    mybir.ActivationFunctionType.Exp,
                                    scale=SCALE)
                            ps_alo = ps_av_tile()
                            ps_ahi = ps_av_tile()
                            for mc in range(MC):
                                nc.tensor.matmul(
                                    ps_alo[:],
                                    v_lo[:, mc, hl * 128:(hl + 1) * 128],
                                    expt[:, mc * 512:(mc + 1) * 512],
                                    start=(mc == 0), stop=(mc == MC - 1))
                            for mc in range(MC):
                                nc.tensor.matmul(
                                    ps_ahi[0:97, :],
                                    v_hi[:, mc, hl * 97:(hl + 1) * 97],
                                    expt[:, mc * 512:(mc + 1) * 512],
                                    start=(mc == 0), stop=(mc == MC - 1))
                            # denom (psum row 96) -> partition-0 sbuf row
                            # (custom-DVE ops ignore the AP partition
                            # offset, so recip can't read row 96 directly)
                            # -> f32 approx recip (denoms are sums of exps,
                            # far from the undefined edge cases) -> gpsimd
                            # broadcast -> fused normalize drains
                            rraw = nrmp.tile([1, 512], F32, tag="rraw",
                                             name="rraw")
                            nc.vector.tensor_copy(rraw[:], ps_ahi[96:97, :])
                            rrow = nrmp.tile([1, 512], F32, tag="rrow",
                                             name="rrow")
                            nc.vector.reciprocal_approx_fast(
                                rrow[:], rraw[:])
                            bc = nrmp.tile([128, 512], F32, tag="bc",
                                           name="bc")
                            nc.gpsimd.partition_broadcast(bc[:], rrow[:])
                            ntsl = slice(nt * 512, (nt + 1) * 512)
                            nc.vector.tensor_tensor(
                                xav[:, h_abs, ntsl], ps_alo[:], bc[:],
                                mybir.AluOpType.mult)
                            for (sb, ch, db, rows) in _hi_frags(h_abs):
                                nc.vector.tensor_tensor(
                                    xav[db:db + rows, ch, ntsl],
                                    ps_ahi[sb:sb + rows, :],
                                    bc[0:rows, :],
                                    mybir.AluOpType.mult)

                pending_out.append((b, xav))

            while pending_out:
                emit_out_proj()

    nc.compile()
    return nc


def _prep_weights(Wq_lr, Wk_lr, Wv_lr, Wout_lr, Wq_full, Wk_full, Wv_full,
                  Wout_full, b_out_full):
    """Host-side weight concat/transpose into device layouts (bf16)."""
    bf16 = ml_dtypes.bfloat16

    def cat_heads(W_lr, W_full):
        # -> (H, 224, C)
        lr = W_lr.reshape(HEADS, RANK, C)
        fl = W_full.reshape(HEADS, DIM_HEAD, C)
        return np.concatenate([lr, fl], axis=1)

    def slab_lo(Wcat):
        # (H,224,C) -> per group [G, 128(p), CK, HG*128] with layout
        # [g][p, co, hl*128+j] = Wcat[g*HG+hl, j, co*128+p]
        A = Wcat[:, :128, :].reshape(GROUPS, HG, 128, CK, 128)
        return np.ascontiguousarray(A.transpose(0, 4, 3, 1, 2)
                                    .reshape(GROUPS, 128, CK, HG * 128)
                                    ).astype(bf16)

    def slab_hi(Wcat, width, pad_to=None):
        A = Wcat[:, 128:224, :].reshape(GROUPS, HG, 96, CK, 128)
        A = A.transpose(0, 4, 3, 1, 2)  # (G, p, co, hl, 96)
        if pad_to is not None:
            pad = np.zeros(A.shape[:-1] + (pad_to - 96,), A.dtype)
            A = np.concatenate([A, pad], axis=-1)
            width = pad_to
        return np.ascontiguousarray(
            A.reshape(GROUPS, 128, CK, HG * width)).astype(bf16)

    Wq_cat = cat_heads(Wq_lr, Wq_full)
    Wk_cat = cat_heads(Wk_lr, Wk_full)
    Wv_cat = cat_heads(Wv_lr, Wv_full)

    # output projection, packed: Wo_cat (H, 224, C) with
    # Wo_cat[h, d, c] = Wout_cat[c, h*224+d]; packed row id rid(h, d)
    Wo_lr = Wout_lr.reshape(C, HEADS, RANK)
    Wo_fl = Wout_full.reshape(C, HEADS, DIM_HEAD)
    Wo_cat = np.concatenate([Wo_lr, Wo_fl], axis=2).transpose(1, 2, 0)  # (H,224,C)
    wo_pk = np.empty((HEADS * DH, C), np.float32)
    for h in range(HEADS):
        wo_pk[128 * h:128 * (h + 1)] = Wo_cat[h, :128, :]
        wo_pk[1024 + 96 * h:1024 + 96 * (h + 1)] = Wo_cat[h, 128:, :]
    wo_pk = wo_pk.reshape(PK, 128, C).astype(bf16)

    bias = np.ascontiguousarray(
        b_out_full.reshape(CK, 128).T).astype(np.float32)  # [128, CK]

    return {
        "wq_lo": slab_lo(Wq_cat), "wq_hi": slab_hi(Wq_cat, 96),
        "wk_lo": slab_lo(Wk_cat), "wk_hi": slab_hi(Wk_cat, 96),
        "wv_lo": slab_lo(Wv_cat), "wv_hi": slab_hi(Wv_cat, 96, pad_to=97),
        "wo_pk": wo_pk, "bias": bias,
    }


def _prep_xt(hs_core):
    # (BL, N, C) f32 -> [128, CK, BL*N] bf16, xt[p, co, b*N+n] = x[b, n, co*128+p]
    X = hs_core.reshape(BL * N, CK, 128).transpose(2, 1, 0)
    return np.ascontiguousarray(X).astype(ml_dtypes.bfloat16)


_NC_CACHE = {}


def get_nc():
    if "nc" not in _NC_CACHE:
        _NC_CACHE["nc"] = build_bass()
    return _NC_CACHE["nc"]


def kernel(hidden_states, Wq_lr, Wk_lr, Wv_lr, Wout_lr,
           Wq_full, Wk_full, Wv_full, Wout_full, b_out_full):
    hidden_states = np.asarray(hidden_states, np.float32)
    weights = _prep_weights(
        np.asarray(Wq_lr, np.float32), np.asarray(Wk_lr, np.float32),
        np.asarray(Wv_lr, np.float32), np.asarray(Wout_lr, np.float32),
        np.asarray(Wq_full, np.float32), np.asarray(Wk_full, np.float32),
        np.asarray(Wv_full, np.float32), np.asarray(Wout_full, np.float32),
        np.asarray(b_out_full, np.float32))

    in_maps = []
    for c in range(NCORES):
        m = dict(weights)
        m["xt"] = _prep_xt(hidden_states[c * BL:(c + 1) * BL])
        in_maps.append(m)

    nc = get_nc()
    results = run_bass_kernel_spmd(nc, in_maps, core_ids=list(range(NCORES))).results

    out = np.empty((B, N, C), np.float32)
    for c in range(NCORES):
        o = results[c]["out"].astype(np.float32)  # (BL, CK, 128, N)
        out[c * BL:(c + 1) * BL] = (
            o.transpose(0, 3, 1, 2).reshape(BL, N, C))
    return out


if __name__ == "__main__":
    nc = get_nc()
    print("built + compiled OK")


# revision 22
# speedup vs baseline: 1.0056x; 1.0025x over previous
"""Trainium2 Bass kernel for dual-branch (low-rank + full-rank) self-attention.

Math (per batch b, head h):
  q = x @ Wq_cat[h].T   (N, 224)   224 = 64 (lr) + 160 (full)
  scoresT[m, n] = sum_d K[m, d] Q[n, d]           (keys m on partitions)
  expT = exp(SCALE * scoresT)                     (no max subtraction; f32 psum)
  xav[d, n] = sum_m Vaug[m, d] expT[m, n]         Vaug has a ones column ->
                                                  psum row 96 of hi = denom
  recip = 1/denom (f32 approx-fast row), partition_broadcast on gpsimd,
  and the AV psum->sbuf drains multiply by it (fused normalize) into a
  PACKED xav:
    packed row id: rid(h, d) = 128h + d        (d < 128, "lo")
                   rid(h, d) = 1024 + 96h + (d-128)   (d >= 128, "hi")
  so the output projection contracts exactly 14 full 128-row chunks
  (14 matmuls per output tile instead of 16); bias is added by the
  scalar-engine drain (per-partition bias vector).

Sharding: data-parallel, 2 batches per core across 8 cores. No collectives.
All matmuls bf16 with f32 PSUM accumulation; softmax in f32 psum.
"""

import os
import sys

sys.path.insert(0, "/opt/trn_rl_repo")

import numpy as np
import ml_dtypes

import concourse.bass as bass
import concourse.mybir as mybir
import concourse.tile as tile
from concourse import bacc
from concourse.bass_utils import run_bass_kernel_spmd

# problem constants (hardcoded per spec)
B, N, C = 16, 1024, 1280
HEADS = 8
RANK = 64
DIM_HEAD = 160
DH = RANK + DIM_HEAD          # 224 concat head dim
SCALE = DIM_HEAD ** (-0.5)
NCORES = 8
BL = B // NCORES              # batches per core = 2
CK = C // 128                 # 10 contraction chunks over C
GROUPS = 2                    # head groups per core pass
HG = HEADS // GROUPS          # 4 heads per group
MC = N // 128                 # 8 key chunks
NT = N // 512                 # 2 query-column tiles
PK = (HEADS * DH) // 128      # 14 packed out-proj contraction chunks

BF16 = mybir.dt.bfloat16
F32 = mybir.dt.float32


def _hi_frags(h):
    """Fragment list for copying AV-hi psum rows 0:96 of head h into the
    packed xav chunks. Returns [(src_base, chunk, dst_base, rows)] where
    every operand window is legal: base 0 spans freely, other bases are
    32-aligned with rows <= 32."""
    frags = []
    start = 1024 + 96 * h
    src = 0
    while src < 96:
        rid = start + src
        chunk, db = divmod(rid, 128)
        rows = min(96 - src, 128 - db)
        if db != 0 or src != 0:
            rows = min(rows, 32)
        frags.append((src, chunk, db, rows))
        src += rows
    return frags


# out-proj accumulation order: the chunks fed by the last head's drains
# (7 = h7 lo, 13 = h5-7 hi) go last so the tail's final projection has
# maximal PE work ready before those drains land
PK_ORDER = [0, 1, 2, 3, 8, 9, 10, 4, 5, 6, 11, 12, 7, 13]


def build_bass():
    nc = bacc.Bacc("TRN2", target_bir_lowering=False, debug=False,
                   num_devices=NCORES)

    def din(name, shape, dt=BF16):
        return nc.dram_tensor(name, shape, dt, kind="ExternalInput").ap()

    xt_d = din("xt", [128, CK, BL * N])                 # x transposed, c-major
    wq_lo_d = din("wq_lo", [GROUPS, 128, CK, HG * 128])
    wq_hi_d = din("wq_hi", [GROUPS, 128, CK, HG * 96])
    wk_lo_d = din("wk_lo", [GROUPS, 128, CK, HG * 128])
    wk_hi_d = din("wk_hi", [GROUPS, 128, CK, HG * 96])
    wv_lo_d = din("wv_lo", [GROUPS, 128, CK, HG * 128])
    wv_hi_d = din("wv_hi", [GROUPS, 128, CK, HG * 97])  # 97th col zero (ones col)
    wo_pk_d = din("wo_pk", [PK, 128, C])                # packed out-proj weights
    bias_d = din("bias", [128, CK], mybir.dt.float32)   # bias[ct*128+p] at [p, ct]
    out_d = nc.dram_tensor("out", [BL, CK, 128, N], BF16, kind="ExternalOutput").ap()

    with tile.TileContext(nc) as tc:
        with (
            tc.tile_pool(name="xtp", bufs=1) as xtp,
            tc.tile_pool(name="wp", bufs=5) as wp,
            tc.tile_pool(name="wop", bufs=PK + 1) as wop,
            tc.tile_pool(name="qkvp", bufs=1) as qkvp,
            tc.tile_pool(name="xavp", bufs=1) as xavp,
            tc.tile_pool(name="expp", bufs=1) as expp,
            tc.tile_pool(name="nrmp", bufs=2) as nrmp,
            tc.tile_pool(name="outp", bufs=3) as outp,
            tc.tile_pool(name="psp", bufs=4, space="PSUM") as psp,
            tc.tile_pool(name="psavp", bufs=4, space="PSUM") as psavp,
        ):
            def ps_tile():
                # general-purpose matmul accumulators (scores/proj/out-proj)
                return psp.tile([128, 512], F32, tag="mm", name="ps")

            def ps_av_tile():
                # AV accumulators — separate slots so the normalize/drain
                # chain never blocks the next head's scores
                return psavp.tile([128, 512], F32, tag="av", name="ps_av")

            wo_t = []          # persistent packed out-proj weight tiles
            bias_t = None
            pending_out = []

            def emit_out_proj():
                # output projection (c on partitions; host untransposes)
                ob, oxav = pending_out.pop(0)
                for ct in range(CK):
                    for nt in range(NT):
                        ps_o = ps_tile()
                        for i, pk in enumerate(PK_ORDER):
                            nc.tensor.matmul(
                                ps_o[:],
                                wo_t[pk][:, ct * 128:(ct + 1) * 128],
                                oxav[:, pk, nt * 512:(nt + 1) * 512],
                                start=(i == 0), stop=(i == PK - 1))
                        ot = outp.tile([128, 512], BF16, tag="ot", name="ot")
                        nc.scalar.activation(
                            ot[:], ps_o[:],
                            mybir.ActivationFunctionType.Identity,
                            bias=bias_t[:, ct:ct + 1], scale=1.0)
                        nc.sync.dma_start(
                            out_d[ob, ct, :, nt * 512:(nt + 1) * 512], ot[:])

            for b in range(BL):
                xt = xtp.tile([128, CK, N], BF16, tag="xt")
                if b > 0:
                    for co in range(CK):
                        nc.sync.dma_start(xt[:, co, :],
                                          xt_d[:, co, b * N:(b + 1) * N])

                xav = xavp.tile([128, PK, N], BF16, tag="xav")

                for g in range(GROUPS):
                    # ---- stream this group's projection weights ----
                    def slab(width):
                        return wp.tile([128, CK, width], BF16, tag="wslab",
                                       name="wslab")
                    w_qlo = slab(HG * 128)
                    w_qhi = slab(HG * 96)
                    w_klo = slab(HG * 128)
                    w_khi = slab(HG * 96)
                    w_vlo = slab(HG * 128)
                    w_vhi = slab(HG * 97)

                    def load_chunk(t, dram, co):
                        nc.sync.dma_start(t[:, co, :], dram[g, :, co, :])

                    qk_slabs = [(w_qlo, wq_lo_d), (w_qhi, wq_hi_d),
                                (w_klo, wk_lo_d), (w_khi, wk_hi_d)]
                    v_slabs = [(w_vlo, wv_lo_d), (w_vhi, wv_hi_d)]
                    if b == 0 and g == 0:
                        # startup: the critical stream is xt + wq_lo — issue
                        # those interleaved first, the rest behind them
                        for co in range(CK):
                            load_chunk(w_qlo, wq_lo_d, co)
                            nc.sync.dma_start(xt[:, co, :],
                                              xt_d[:, co, 0:N])
                        for co in range(CK):
                            for t, dram in qk_slabs[1:]:
                                load_chunk(t, dram, co)
                        for co in range(CK):
                            for t, dram in v_slabs:
                                load_chunk(t, dram, co)
                    else:
                        for t, dram in qk_slabs + v_slabs:
                            for co in range(CK):
                                load_chunk(t, dram, co)

                    # ---- projections: qT/kT (head-dim on partitions) ----
                    qt_lo = qkvp.tile([128, HG, N], BF16, tag="qt_lo")
                    qt_hi = qkvp.tile([128, HG, N], BF16, tag="qt_hi")
                    kt_lo = qkvp.tile([128, HG, N], BF16, tag="kt_lo")
                    kt_hi = qkvp.tile([128, HG, N], BF16, tag="kt_hi")

                    # hi chunks of 4 heads (4x96=384 rows) pack into 3 full
                    # M=128 matmuls; fragments unmixed by the psum->sbuf
                    # copies (pieces respect the 32-aligned partition-window
                    # rule: base 0 may span freely, other bases max 32 rows)
                    HI_FRAGS = {  # ch -> [(hl, src_base, dst_base, rows)]
                        0: [(0, 0, 0, 96), (1, 96, 0, 32)],
                        1: [(1, 0, 32, 32), (1, 32, 64, 32),
                            (2, 64, 0, 32), (2, 96, 32, 32)],
                        2: [(2, 0, 64, 32), (3, 32, 0, 32),
                            (3, 64, 32, 32), (3, 96, 64, 32)],
                    }
                    def proj_lo(wlo, tlo, co_outer, scalar_drain=False):
                        units = [(hl, nt) for hl in range(HG)
                                 for nt in range(NT)]
                        drain_copy = (nc.scalar.copy if scalar_drain
                                      else nc.vector.tensor_copy)
                        if co_outer:
                            # all 8 psum banks accumulate at once so each
                            # arriving DMA chunk feeds 8 matmuls immediately
                            # (otherwise the single open chain is DMA-paced
                            # during startup)
                            pss = ([ps_tile() for _ in range(4)]
                                   + [ps_av_tile() for _ in range(4)])
                            for co in range(CK):
                                for i, (hl, nt) in enumerate(units):
                                    nc.tensor.matmul(
                                        pss[i][:],
                                        wlo[:, co, hl * 128:(hl + 1) * 128],
                                        xt[:, co, nt * 512:(nt + 1) * 512],
                                        start=(co == 0), stop=(co == CK - 1))
                            for i, (hl, nt) in enumerate(units):
                                # scalar engine: it is idle until the first
                                # exps (~60us in), and this keeps the DVE
                                # queue clear of the co-outer drain burst so
                                # later projections get psum slots promptly
                                nc.scalar.copy(
                                    tlo[:, hl, nt * 512:(nt + 1) * 512],
                                    pss[i][:])
                        else:
                            for (hl, nt) in units:
                                ps = ps_tile()
                                for co in range(CK):
                                    nc.tensor.matmul(
                                        ps[:],
                                        wlo[:, co, hl * 128:(hl + 1) * 128],
                                        xt[:, co, nt * 512:(nt + 1) * 512],
                                        start=(co == 0), stop=(co == CK - 1))
                                drain_copy(
                                    tlo[:, hl, nt * 512:(nt + 1) * 512], ps[:])

                    def proj_hi(whi, thi, co_outer):
                        units = [(ch, nt) for ch in range(3)
                                 for nt in range(NT)]

                        def drain(ps, ch, nt):
                            for (hl, sb, db, rows) in HI_FRAGS[ch]:
                                nc.vector.tensor_copy(
                                    thi[db:db + rows, hl,
                                        nt * 512:(nt + 1) * 512],
                                    ps[sb:sb + rows, :])
                        if co_outer:
                            pss = ([ps_tile() for _ in range(4)]
                                   + [ps_av_tile() for _ in range(2)])
                            for co in range(CK):
                                for i, (ch, nt) in enumerate(units):
                                    nc.tensor.matmul(
                                        pss[i][:],
                                        whi[:, co, ch * 128:(ch + 1) * 128],
                                        xt[:, co, nt * 512:(nt + 1) * 512],
                                        start=(co == 0), stop=(co == CK - 1))
                            for i, (ch, nt) in enumerate(units):
                                drain(pss[i], ch, nt)
                        else:
                            for (ch, nt) in units:
                                ps = ps_tile()
                                for co in range(CK):
                                    nc.tensor.matmul(
                                        ps[:],
                                        whi[:, co, ch * 128:(ch + 1) * 128],
                                        xt[:, co, nt * 512:(nt + 1) * 512],
                                        start=(co == 0), stop=(co == CK - 1))
                                drain(ps, ch, nt)

                    startup = (b == 0 and g == 0)
                    proj_lo(w_qlo, qt_lo, startup)
                    proj_hi(w_qhi, qt_hi, startup)
                    proj_lo(w_klo, kt_lo, False,
                            scalar_drain=startup)
                    proj_hi(w_khi, kt_hi, False)

                    # ---- V projection: natural layout (keys on partitions) ----
                    v_lo = qkvp.tile([128, MC, HG * 128], BF16, tag="v_lo")
                    v_hi = qkvp.tile([128, MC, HG * 97], BF16, tag="v_hi")
                    for mc in range(MC):
                        ps_l = ps_tile()
                        ps_h = ps_tile()
                        for co in range(CK):
                            nc.tensor.matmul(
                                ps_l[:],
                                xt[:, co, mc * 128:(mc + 1) * 128],
                                w_vlo[:, co, :],
                                start=(co == 0), stop=(co == CK - 1))
                        for co in range(CK):
                            nc.tensor.matmul(
                                ps_h[:, 0:HG * 97],
                                xt[:, co, mc * 128:(mc + 1) * 128],
                                w_vhi[:, co, :],
                                start=(co == 0), stop=(co == CK - 1))
                        nc.vector.tensor_copy(v_lo[:, mc, :], ps_l[:])
                        nc.vector.tensor_copy(v_hi[:, mc, :], ps_h[:, 0:HG * 97])
                    # ones column per head (softmax denominator row source)
                    for hl in range(HG):
                        nc.gpsimd.memset(v_hi[:, :, hl * 97 + 96], 1.0)

                    # persistent out-proj weights: emit once, during b0/g0
                    # compute (keeps the startup DMA window clear)
                    if b == 0 and g == 0:
                        for pk in range(PK):
                            wt = wop.tile([128, C], BF16, tag="wo", name="wo")
                            nc.sync.dma_start(wt[:], wo_pk_d[pk])
                            wo_t.append(wt)
                        bias_t = wop.tile([128, CK], F32, tag="bias",
                                          name="bias")
                        nc.sync.dma_start(bias_t[:], bias_d)

                    # previous batch's output projection: emitted while this
                    # group's dense matmuls keep the PE busy
                    if g == 0 and pending_out:
                        emit_out_proj()

                    # ---- attention per head: AV + fused normalize drains ----
                    for hl in range(HG):
                        h_abs = g * HG + hl
                        for nt in range(NT):
                            expt = expp.tile([128, MC * 512], BF16, tag="expt")
                            for mc in range(MC):
                                ps_sc = ps_tile()
                                nc.tensor.matmul(
                                    ps_sc[:],
                                    kt_lo[:, hl, mc * 128:(mc + 1) * 128],
                                    qt_lo[:, hl, nt * 512:(nt + 1) * 512],
                                    start=True, stop=False)
                                nc.tensor.matmul(
                                    ps_sc[:],
                                    kt_hi[0:96, hl, mc * 128:(mc + 1) * 128],
                                    qt_hi[0:96, hl, nt * 512:(nt + 1) * 512],
                                    start=False, stop=True)
                                nc.scalar.activation(
                                    expt[:, mc * 512:(mc + 1) * 512],
                                    ps_sc[:],
                                    mybir.ActivationFunctionType.Exp,
                                    scale=SCALE)
                            ps_alo = ps_av_tile()
                            ps_ahi = ps_av_tile()
                            for mc in range(MC):
                                nc.tensor.matmul(
                                    ps_alo[:],
                                    v_lo[:, mc, hl * 128:(hl + 1) * 128],
                                    expt[:, mc * 512:(mc + 1) * 512],
                                    start=(mc == 0), stop=(mc == MC - 1))
                            for mc in range(MC):
                                nc.tensor.matmul(
                                    ps_ahi[0:97, :],
                                    v_hi[:, mc, hl * 97:(hl + 1) * 97],
                                    expt[:, mc * 512:(mc + 1) * 512],
                                    start=(mc == 0), stop=(mc == MC - 1))
                            # denom (psum row 96) -> partition-0 sbuf row
                            # (custom-DVE ops ignore the AP partition
                            # offset, so recip can't read row 96 directly)
                            # -> f32 approx recip (denoms are sums of exps,
                            # far from the undefined edge cases) -> gpsimd
                            # broadcast -> fused normalize drains
                            rraw = nrmp.tile([1, 512], F32, tag="rraw",
                                             name="rraw")
                            nc.vector.tensor_copy(rraw[:], ps_ahi[96:97, :])
                            rrow = nrmp.tile([1, 512], F32, tag="rrow",
                                             name="rrow")
                            nc.vector.reciprocal_approx_fast(
                                rrow[:], rraw[:])
                            bc = nrmp.tile([128, 512], F32, tag="bc",
                                           name="bc")
                            nc.gpsimd.partition_broadcast(bc[:], rrow[:])
                            ntsl = slice(nt * 512, (nt + 1) * 512)
                            nc.vector.tensor_tensor(
                                xav[:, h_abs, ntsl], ps_alo[:], bc[:],
                                mybir.AluOpType.mult)
                            for (sb, ch, db, rows) in _hi_frags(h_abs):
                                nc.vector.tensor_tensor(
                                    xav[db:db + rows, ch, ntsl],
                                    ps_ahi[sb:sb + rows, :],
                                    bc[0:rows, :],
                                    mybir.AluOpType.mult)

                pending_out.append((b, xav))

            while pending_out:
                emit_out_proj()

    nc.compile()
    return nc


def _prep_weights(Wq_lr, Wk_lr, Wv_lr, Wout_lr, Wq_full, Wk_full, Wv_full,
                  Wout_full, b_out_full):
    """Host-side weight concat/transpose into device layouts (bf16)."""
    bf16 = ml_dtypes.bfloat16

    def cat_heads(W_lr, W_full):
        # -> (H, 224, C)
        lr = W_lr.reshape(HEADS, RANK, C)
        fl = W_full.reshape(HEADS, DIM_HEAD, C)
        return np.concatenate([lr, fl], axis=1)

    def slab_lo(Wcat):
        # (H,224,C) -> per group [G, 128(p), CK, HG*128] with layout
        # [g][p, co, hl*128+j] = Wcat[g*HG+hl, j, co*128+p]
        A = Wcat[:, :128, :].reshape(GROUPS, HG, 128, CK, 128)
        return np.ascontiguousarray(A.transpose(0, 4, 3, 1, 2)
                                    .reshape(GROUPS, 128, CK, HG * 128)
                                    ).astype(bf16)

    def slab_hi(Wcat, width, pad_to=None):
        A = Wcat[:, 128:224, :].reshape(GROUPS, HG, 96, CK, 128)
        A = A.transpose(0, 4, 3, 1, 2)  # (G, p, co, hl, 96)
        if pad_to is not None:
            pad = np.zeros(A.shape[:-1] + (pad_to - 96,), A.dtype)
            A = np.concatenate([A, pad], axis=-1)
            width = pad_to
        return np.ascontiguousarray(
            A.reshape(GROUPS, 128, CK, HG * width)).astype(bf16)

    Wq_cat = cat_heads(Wq_lr, Wq_full)
    Wk_cat = cat_heads(Wk_lr, Wk_full)
    Wv_cat = cat_heads(Wv_lr, Wv_full)

    # output projection, packed: Wo_cat (H, 224, C) with
    # Wo_cat[h, d, c] = Wout_cat[c, h*224+d]; packed row id rid(h, d)
    Wo_lr = Wout_lr.reshape(C, HEADS, RANK)
    Wo_fl = Wout_full.reshape(C, HEADS, DIM_HEAD)
    Wo_cat = np.concatenate([Wo_lr, Wo_fl], axis=2).transpose(1, 2, 0)  # (H,224,C)
    wo_pk = np.empty((HEADS * DH, C), np.float32)
    for h in range(HEADS):
        wo_pk[128 * h:128 * (h + 1)] = Wo_cat[h, :128, :]
        wo_pk[1024 + 96 * h:1024 + 96 * (h + 1)] = Wo_cat[h, 128:, :]
    wo_pk = wo_pk.reshape(PK, 128, C).astype(bf16)

    bias = np.ascontiguousarray(
        b_out_full.reshape(CK, 128).T).astype(np.float32)  # [128, CK]

    return {
        "wq_lo": slab_lo(Wq_cat), "wq_hi": slab_hi(Wq_cat, 96),
        "wk_lo": slab_lo(Wk_cat), "wk_hi": slab_hi(Wk_cat, 96),
        "wv_lo": slab_lo(Wv_cat), "wv_hi": slab_hi(Wv_cat, 96, pad_to=97),
        "wo_pk": wo_pk, "bias": bias,
    }


def _prep_xt(hs_core):
    # (BL, N, C) f32 -> [128, CK, BL*N] bf16, xt[p, co, b*N+n] = x[b, n, co*128+p]
    X = hs_core.reshape(BL * N, CK, 128).transpose(2, 1, 0)
    return np.ascontiguousarray(X).astype(ml_dtypes.bfloat16)


_NC_CACHE = {}


def get_nc():
    if "nc" not in _NC_CACHE:
        _NC_CACHE["nc"] = build_bass()
    return _NC_CACHE["nc"]


def kernel(hidden_states, Wq_lr, Wk_lr, Wv_lr, Wout_lr,
           Wq_full, Wk_full, Wv_full, Wout_full, b_out_full):
    hidden_states = np.asarray(hidden_states, np.float32)
    weights = _prep_weights(
        np.asarray(Wq_lr, np.float32), np.asarray(Wk_lr, np.float32),
        np.asarray(Wv_lr, np.float32), np.asarray(Wout_lr, np.float32),
        np.asarray(Wq_full, np.float32), np.asarray(Wk_full, np.float32),
        np.asarray(Wv_full, np.float32), np.asarray(Wout_full, np.float32),
        np.asarray(b_out_full, np.float32))

    in_maps = []
    for c in range(NCORES):
        m = dict(weights)
        m["xt"] = _prep_xt(hidden_states[c * BL:(c + 1) * BL])
        in_maps.append(m)

    nc = get_nc()
    results = run_bass_kernel_spmd(nc, in_maps, core_ids=list(range(NCORES))).results

    out = np.empty((B, N, C), np.float32)
    for c in range(NCORES):
        o = results[c]["out"].astype(np.float32)  # (BL, CK, 128, N)
        out[c * BL:(c + 1) * BL] = (
            o.transpose(0, 3, 1, 2).reshape(BL, N, C))
    return out


if __name__ == "__main__":
    nc = get_nc()
    print("built + compiled OK")
